# revision 1
# baseline (speedup 1.0000x reference)
"""Cross-attention Trainium2 kernel (8 NeuronCores, batch-data-parallel).

Computes, per batch element b:
    q = x[b] @ Wq            [S, DK]
    k = y[b] @ Wk            [S, DK]
    v = y[b] @ Wv            [S, E]
    p = exp((q @ k.T) / sqrt(E))        (no max-subtraction: logits ~ N(0, .25))
    out[b] = (p @ v) / rowsum(p) + x[b]

Layout strategy (per core, BL=2 batches):
  - All matmuls run in fp8e4 with perf_mode=DoubleRow (2 fp8 weights/cell,
    2 MACs/cycle): operands are stored as [128, 2, free] "k-pair" tiles so a
    single matmul contracts 256 elements; PSUM accumulates fp32.
  - Weights are pre-scaled by 8 when cast to fp8 (keeps N(0,1/1024) entries
    out of the fp8 subnormal range); the score scale folds the 8*8 back out,
    and the rowsum ones-column is 8.0 so the softmax normalization of the
    8x-scaled v cancels exactly.
  - Activations are transposed on-chip (cast-DMA fp32->bf16 into a DRAM
    bounce, then xbar DMA-transpose into SBUF bf16, then DVE cast to the
    fp8 k-pair tiles) so the contraction dim of every matmul sits on
    partitions:
        xT, yT : [C, S]     qT = Wq.T @ xT : [DK, S]   kT : [DK, S]
        v  = yT.T @ Wv : [S_kv, E]  (natural layout)
        sT = kT.T @ qT : [S_kv, S_q]   (scoresT; softmax axis = partitions)
        pT = exp(sT*scale)             (stationary of the AV matmul)
        out = pT.T @ [v | 8]           (8s column yields 8*rowsum(p) free)
  - Epilogue fuses (psum * 1/rowsum8) + x in one DVE scalar_tensor_tensor.
"""

import math

import numpy as np

# Full-problem constants (hardcoded per the harness contract).
B_FULL = 16
N_CORES = 8
S_Q = 2048
S_KV = 2048
C_DIM = 1024  # input feature dim (contraction of the projections)
DK = 256  # q/k head dim
E_DIM = 1024  # v / output dim
P = 128
WSC = 8.0  # fp8 pre-scale on Wq/Wk/Wv (and the rowsum ones column)


class CFG:
    def __init__(self, bl, sq, skv, c, dk, e, s_block=None, n_free=512):
        assert sq % P == 0 and skv % P == 0 and c % P == 0 and dk % P == 0
        self.bl = bl  # batches per core
        self.sq = sq
        self.skv = skv
        self.c = c
        self.dk = dk
        self.e = e
        self.s_block = s_block or min(1024, sq)  # query cols processed per wave
        assert sq % self.s_block == 0
        self.n_free = n_free  # moving-operand free-dim per matmul
        self.scale = 1.0 / math.sqrt(e)


def _chunks(total, size):
    out = []
    o = 0
    while o < total:
        out.append((o, min(size, total - o)))
        o += size
    return out


def emit_cross_attention(tc, outs, ins, cfg):
    """Emit the kernel into TileContext `tc`.

    outs/ins are dicts of DRAM APs: ins = x, y, Wq, Wk, Wv ; outs = out.
    x/y/out: [bl, sq|skv, c|e] fp32. Weights: [c, dk|e] fp32.
    """
    import concourse.mybir as mybir
    from concourse.mybir import ActivationFunctionType as AF
    from concourse.mybir import AluOpType as ALU
    from concourse.tile_rust import add_dep_helper

    DR = mybir.MatmulPerfMode.DoubleRow

    nc = tc.nc
    bf16 = mybir.dt.bfloat16
    fp8 = mybir.dt.float8e4
    f32 = mybir.dt.float32

    x, y, Wq, Wk, Wv = ins["x"], ins["y"], ins["Wq"], ins["Wk"], ins["Wv"]
    out = outs["out"]

    n_ct = cfg.c // P  # 128-contraction tiles of the projections
    n_cg = n_ct // 2  # DoubleRow (256-contraction) groups of the projections
    n_tt = cfg.skv // P  # key/value 128-tiles
    n_tg = n_tt // 2  # key/value DoubleRow groups (AV contraction)
    n_dt = cfg.dk // P  # qk-dim 128-tiles (score contraction; must be 2)
    assert n_dt == 2, "scores assume DK == 256 (one DoubleRow group)"
    s_waves = _chunks(cfg.sq, cfg.s_block)

    # DRAM bounce buffers for the bf16 copies of x and y (per local batch).
    xb = nc.dram_tensor("xb16", [cfg.bl, cfg.sq, cfg.c], bf16).ap()
    yb = nc.dram_tensor("yb16", [cfg.bl, cfg.skv, cfg.c], bf16).ap()

    pool = tc.alloc_tile_pool(name="main", bufs=1)
    ps_mm = tc.alloc_tile_pool(name="ps_mm", bufs=2, space="PSUM")
    ps_av = tc.alloc_tile_pool(name="ps_av", bufs=2, space="PSUM")

    # Measured DMA facts this layout is built on:
    #   - SWDGE D2D cast runs at ~360 GB/s payload.
    #   - xbar transposes cost ~1us fixed + ~400 GB/s; they only exist on
    #     one ring (concurrent transposes on both HWDGE rings corrupt), and
    #     Tile serializes every transpose group against ALL in-flight DMAs.
    #     So the global stream alternates copy-windows and transpose-windows,
    #     ordered here so each window's data is needed just after it closes.
    #   - SWDGE queue: casts only; sync ring: transposes only; scalar ring:
    #     weights / residual / output plain DMAs.
    half = cfg.skv // 2 if cfg.skv >= 1024 else cfg.skv
    y0_chunks = _chunks(cfg.skv, half)

    # fp8 k-pair weight tiles: w8[g][:, j, :] holds rows (2g+j)*128..+128.
    wq8 = [pool.tile([P, 2, cfg.dk], fp8, tag=f"wq{g}", name=f"wq{g}") for g in range(n_cg)]
    wk8 = [pool.tile([P, 2, cfg.dk], fp8, tag=f"wk{g}", name=f"wk{g}") for g in range(n_cg)]
    wv8 = [pool.tile([P, 2, cfg.e], fp8, tag=f"wv{g}", name=f"wv{g}") for g in range(n_cg)]

    def load_weight(which, w_dram, w_tiles, wdim, kc):
        w_f = pool.tile([P, cfg.e], f32, tag="wstage", bufs=2, name=f"wf{which}{kc}")
        nc.scalar.dma_start(out=w_f[:, :wdim], in_=w_dram[kc * P : (kc + 1) * P, :])
        nc.vector.tensor_scalar_mul(w_tiles[kc // 2][:, kc % 2, :], w_f[:, :wdim], WSC)

    # copy-window 0: y0 first half cast (SWDGE) + wk loads (scalar ring)
    nc.gpsimd.dma_start(out=yb[0][0:half, :], in_=y[0][0:half, :])
    for kc in range(n_ct):
        load_weight("k", Wk, wk8, cfg.dk, kc)

    # 8.0 column (fp8 exact): rowsum of p gets the same 8x scale as v.
    ones_col = pool.tile([P, 2, 16], fp8, tag="ones", name="ones")
    nc.gpsimd.memset(ones_col[:], WSC)

    allT = {}
    for b in range(cfg.bl):
        yT = []
        xT = []
        for kc in range(n_ct):
            yT_t = pool.tile([P, cfg.skv], bf16, tag="actT", bufs=2 * n_ct, name=f"yT{kc}")
            yT.append(yT_t)
        for kc in range(n_ct):
            xT_t = pool.tile([P, cfg.sq], bf16, tag="actT", bufs=2 * n_ct, name=f"xT{kc}")
            xT.append(xT_t)
        # fp8 k-pair copies of the transposed activations.
        yT8 = [
            pool.tile([P, 2, cfg.skv], fp8, tag="actT8", bufs=n_ct, name=f"yT8_{g}")
            for g in range(n_cg)
        ]
        xT8 = [
            pool.tile([P, 2, cfg.sq], fp8, tag="actT8", bufs=n_ct, name=f"xT8_{g}")
            for g in range(n_cg)
        ]
        allT[b] = (yT, xT, yT8, xT8)

    def transpose_group(b, which, ro, rn):
        srcb = yb if which == "y" else xb
        dst = allT[b][0] if which == "y" else allT[b][1]
        last = None
        for kc in range(n_ct):
            last = nc.sync.dma_start(
                out=dst[kc][:, ro : ro + rn],
                in_=srcb[b][ro : ro + rn, kc * P : (kc + 1) * P],
                transpose=True,
            )
        return last

    def cast_group(b, which, ro, rn):
        # bf16 [128, rn] tiles -> fp8 k-pair tiles, on DVE (idle engine).
        src = allT[b][0] if which == "y" else allT[b][1]
        dst = allT[b][2] if which == "y" else allT[b][3]
        for kc in range(n_ct):
            nc.vector.tensor_copy(
                dst[kc // 2][:, kc % 2, ro : ro + rn], src[kc][:, ro : ro + rn]
            )

    def pace(waiter, dependee):
        # Real semaphore edge: keeps the next copy-window out of flight until
        # the previous transpose-window drains (Tile serializes any transpose
        # against every in-flight copy, so un-paced casts stall transposes).
        if waiter is not None and dependee is not None:
            add_dep_helper(waiter.ins, dependee.ins, sync=True, reason="pace dma windows")

    # transpose-window: yT(b0) first half
    tg = transpose_group(0, "y", 0, half)
    cast_group(0, "y", 0, half)
    # copy-window: y0 second half + wv, wq loads
    if half < cfg.skv:
        c = nc.gpsimd.dma_start(out=yb[0][half:, :], in_=y[0][half:, :])
        pace(c, tg)
    for kc in range(n_ct):
        load_weight("v", Wv, wv8, cfg.e, kc)
    for kc in range(n_ct):
        load_weight("q", Wq, wq8, cfg.dk, kc)
    if half < cfg.skv:
        tg = transpose_group(0, "y", half, cfg.skv - half)
        cast_group(0, "y", half, cfg.skv - half)
    # copy-window: x0 cast; then xT(b0) transposes
    c = nc.gpsimd.dma_start(out=xb[0][:], in_=x[0][:])
    pace(c, tg)
    tg = transpose_group(0, "x", 0, cfg.sq)
    cast_group(0, "x", 0, cfg.sq)
    allT["last_tg"] = tg
    # b1 chains are emitted inside the batch loop below (their windows land
    # under b0's scores/AV compute).

    for b in range(cfg.bl):
        yT, xT, yT8, xT8 = allT[b]
        if b > 0:
            c = nc.gpsimd.dma_start(out=yb[b][:], in_=y[b][:])
            pace(c, allT["last_tg"])
            tg = transpose_group(b, "y", 0, cfg.skv)
            cast_group(b, "y", 0, cfg.skv)
            c = nc.gpsimd.dma_start(out=xb[b][:], in_=x[b][:])
            pace(c, tg)
            pace(c, allT.get(f"wave_mid_{b - 1}_0"))
            tg = transpose_group(b, "x", 0, cfg.sq)
            cast_group(b, "x", 0, cfg.sq)
            allT["last_tg"] = tg

        # --- projections: kT/v aligned to the y halves, then qT ------------
        # kT/qT: [128, 2, S] k-pair tiles (DK=256 = one DoubleRow group).
        kT8 = pool.tile([P, 2, cfg.skv], fp8, tag="kT", name="kT8")
        qT8 = pool.tile([P, 2, cfg.sq], fp8, tag="qT", name="qT8")
        # v: k-pair tiles over the AV contraction (t): v8[g][:, j, :].
        v8 = [
            pool.tile([P, 2, cfg.e], fp8, tag="v", bufs=n_tg, name=f"v{g}")
            for g in range(n_tg)
        ]

        for ro, rn in y0_chunks:
            for no, nn_ in _chunks(rn, cfg.n_free):
                for md in range(n_dt):
                    ps = ps_mm.tile([P, cfg.n_free], f32, tag="mm", name="ps_p")
                    for g in range(n_cg):
                        nc.tensor.matmul(
                            ps[:, :nn_],
                            wk8[g][:, :, md * P : (md + 1) * P],
                            yT8[g][:, :, ro + no : ro + no + nn_],
                            start=(g == 0),
                            stop=(g == n_cg - 1),
                            perf_mode=DR,
                        )
                    nc.scalar.activation(
                        kT8[:, md, ro + no : ro + no + nn_], ps[:, :nn_], AF.Copy
                    )
            for mt in range(ro // P, (ro + rn) // P):
                for no, nn_ in _chunks(cfg.e, cfg.n_free):
                    ps = ps_mm.tile([P, cfg.n_free], f32, tag="mm", name="ps_v")
                    for g in range(n_cg):
                        nc.tensor.matmul(
                            ps[:, :nn_],
                            yT8[g][:, :, mt * P : (mt + 1) * P],
                            wv8[g][:, :, no : no + nn_],
                            start=(g == 0),
                            stop=(g == n_cg - 1),
                            perf_mode=DR,
                        )
                    nc.scalar.activation(
                        v8[mt // 2][:, mt % 2, no : no + nn_], ps[:, :nn_], AF.Copy
                    )
        for no, nn_ in _chunks(cfg.sq, cfg.n_free):
            for md in range(n_dt):
                ps = ps_mm.tile([P, cfg.n_free], f32, tag="mm", name="ps_q")
                for g in range(n_cg):
                    nc.tensor.matmul(
                        ps[:, :nn_],
                        wq8[g][:, :, md * P : (md + 1) * P],
                        xT8[g][:, :, no : no + nn_],
                        start=(g == 0),
                        stop=(g == n_cg - 1),
                        perf_mode=DR,
                    )
                nc.scalar.activation(
                    qT8[:, md, no : no + nn_], ps[:, :nn_], AF.Copy
                )

        # --- attention, one wave of s_block query columns at a time --------
        # Scores carry the 8*8 weight pre-scale; exp folds it back out.
        s_scale = cfg.scale / (WSC * WSC)
        for wo, wn in s_waves:
            # scoresT + exp: pT8[g][:, j, s_block] (t-tile 2g+j)
            pT8 = [
                pool.tile([P, 2, cfg.s_block], fp8, tag="pT", bufs=n_tg, name=f"pT{g}")
                for g in range(n_tg)
            ]
            for t in range(n_tt):
                for no, nn_ in _chunks(wn, cfg.n_free):
                    ps = ps_mm.tile([P, cfg.n_free], f32, tag="mm", name="ps_s")
                    nc.tensor.matmul(
                        ps[:, :nn_],
                        kT8[:, :, t * P : (t + 1) * P],
                        qT8[:, :, wo + no : wo + no + nn_],
                        start=True,
                        stop=True,
                        perf_mode=DR,
                    )
                    nc.scalar.activation(
                        pT8[t // 2][:, t % 2, no : no + nn_],
                        ps[:, :nn_],
                        AF.Exp,
                        scale=s_scale,
                    )

            # AV + rowsum + epilogue, per 128-row block of queries
            for mh in range(wn // P):
                sm = wo + mh * P  # global query row offset
                ps_e = ps_av.tile([P, cfg.e], f32, tag="av_e", name="ps_e")
                ps_sum = ps_av.tile([P, 1], f32, tag="av_s", name="ps_sum")
                e_chunks = _chunks(cfg.e, cfg.n_free)
                for g in range(n_tg):
                    lhsT = pT8[g][:, :, mh * P : (mh + 1) * P]
                    for no, nn_ in e_chunks:
                        nc.tensor.matmul(
                            ps_e[:, no : no + nn_],
                            lhsT,
                            v8[g][:, :, no : no + nn_],
                            start=(g == 0),
                            stop=(g == n_tg - 1),
                            perf_mode=DR,
                        )
                    nc.tensor.matmul(
                        ps_sum[:],
                        lhsT,
                        ones_col[:, :, 0:1],
                        start=(g == 0),
                        stop=(g == n_tg - 1),
                        perf_mode=DR,
                    )
                recip = pool.tile([P, 1], f32, tag="recip", bufs=8, name="recip")
                nc.vector.reciprocal(recip[:], ps_sum[:])
                # residual from the bf16 bounce: halves HBM vs re-reading x
                # fp32, and bf16 rounding (~0.2%) is far inside tolerance.
                xres = pool.tile([P, cfg.e], bf16, tag="xres", bufs=6, name="xres")
                nc.scalar.dma_start(out=xres[:], in_=xb[b][sm : sm + P, :])
                out_t = pool.tile([P, cfg.e], f32, tag="out_t", bufs=8, name="out_t")
                nc.vector.scalar_tensor_tensor(
                    out_t[:], ps_e[:], recip[:], xres[:], ALU.mult, ALU.add
                )
                st = nc.scalar.dma_start(out=out[b][sm : sm + P, :], in_=out_t[:])
                if mh == 1:
                    allT[f"wave_mid_{b}_{wo}"] = st
                allT[f"wave_end_{b}_{wo}"] = st

    ps_av.release()
    ps_mm.release()
    pool.release()


def make_tile_kernel(cfg):
    """Adapter with the (tc, outs, ins) signature used by run_kernel/test.py."""

    def k(tc, outs, ins):
        emit_cross_attention(tc, outs, ins, cfg)

    return k


def _build(cfg):
    import concourse.bacc as bacc
    import concourse.mybir as mybir
    import concourse.tile as tile

    f32 = mybir.dt.float32
    nc = bacc.Bacc(
        "TRN2",
        target_bir_lowering=False,
        debug=False,
        enable_asserts=False,
        num_devices=N_CORES,
    )
    ins = {
        "x": nc.dram_tensor("x", [cfg.bl, cfg.sq, cfg.c], f32, kind="ExternalInput").ap(),
        "y": nc.dram_tensor("y", [cfg.bl, cfg.skv, cfg.c], f32, kind="ExternalInput").ap(),
        "Wq": nc.dram_tensor("Wq", [cfg.c, cfg.dk], f32, kind="ExternalInput").ap(),
        "Wk": nc.dram_tensor("Wk", [cfg.c, cfg.dk], f32, kind="ExternalInput").ap(),
        "Wv": nc.dram_tensor("Wv", [cfg.c, cfg.e], f32, kind="ExternalInput").ap(),
    }
    outs = {
        "out": nc.dram_tensor("out", [cfg.bl, cfg.sq, cfg.e], f32, kind="ExternalOutput").ap()
    }
    with tile.TileContext(nc) as tc:
        emit_cross_attention(tc, outs, ins, cfg)
    nc.compile()
    return nc


_CACHED = {}


def run_on_cores(x, y, Wq, Wk, Wv, trace=False):
    from concourse import bass_utils

    cfg = CFG(B_FULL // N_CORES, S_Q, S_KV, C_DIM, DK, E_DIM)
    key = "full"
    if key not in _CACHED:
        _CACHED[key] = _build(cfg)
    nc = _CACHED[key]

    bl = cfg.bl
    in_maps = [
        {
            "x": np.ascontiguousarray(x[i * bl : (i + 1) * bl]),
            "y": np.ascontiguousarray(y[i * bl : (i + 1) * bl]),
            "Wq": Wq,
            "Wk": Wk,
            "Wv": Wv,
        }
        for i in range(N_CORES)
    ]
    res = bass_utils.run_bass_kernel_spmd(
        nc, in_maps, core_ids=list(range(N_CORES)), trace=trace
    )
    out = np.concatenate([r["out"] for r in res.results], axis=0)
    return out, res


def kernel(x, y, Wq, Wk, Wv):
    x = np.asarray(x, dtype=np.float32)
    y = np.asarray(y, dtype=np.float32)
    Wq = np.asarray(Wq, dtype=np.float32)
    Wk = np.asarray(Wk, dtype=np.float32)
    Wv = np.asarray(Wv, dtype=np.float32)
    out, _ = run_on_cores(x, y, Wq, Wk, Wv, trace=False)
    return out



# revision 6
# speedup vs baseline: 1.2193x; 1.2193x over previous
"""Cross-attention Trainium2 kernel (8 NeuronCores, batch-data-parallel).

Computes, per batch element b:
    q = x[b] @ Wq            [S, DK]
    k = y[b] @ Wk            [S, DK]
    v = y[b] @ Wv            [S, E]
    p = exp((q @ k.T) / sqrt(E))        (no max-subtraction: logits ~ N(0, .25))
    out[b] = (p @ v) / rowsum(p) + x[b]

Layout strategy (per core, BL=2 batches):
  - All matmuls run in fp8e4 with perf_mode=DoubleRow: operands are
    [128, 2, free] "k-pair" tiles contracting 256/instruction; PSUM fp32.
  - Activations go fp32 -> fp8e4 in ONE SWDGE cast-DMA into a DRAM bounce,
    then one xbar DMA-transpose per 512-row chunk ON THE fp8 DATA VIEWED AS
    uint16 PAIRS (halves bounce+transpose HBM traffic vs a bf16 bounce).
    The transposed u16 row r = kc*128+p holds the fp8 pair (d=2r, 2r+1), so
    partition p of group kc carries d = 256*kc + 2p + j  (j = pair index).
    A DVE stride-2 copy de-interleaves into standard k-pair tiles; the
    weights are simply loaded with the matching (p j) row order.
  - Weights are pre-scaled by 8 at fp8-cast time (keeps N(0,1/1024) out of
    fp8 subnormals); the score scale folds 8*8 back out and the 8.0 rowsum
    column cancels the 8x on v.
  - Dataflow: xT chunks first (qT), then per y-chunk: kT chunk, v rows, and
    the full scores panel for those 4 key-tiles (both query waves) — so the
    DMA-bound prologue is packed with PE work and batch 0's attention phase
    is AV-only.  exp runs on ACT; PSUM drains of kT/qT/v on DVE.  rowsum
    rides an 8.0-column matmul into one shared PSUM bank (col = mh).
  - batch 1: bounce casts bulk-stream during batch 0's AV; transposes are
    sprinkled between AV rows (each is a hard barrier vs in-flight DMAs);
    its v-projection interleaves with its scores waves to hide exp.
  - Output is written bf16 and upcast to fp32 on the host; the residual x
    is re-read as fp32 directly from HBM in the epilogue
    (out = psum/rowsum8 + x via one DVE scalar_tensor_tensor).
"""

import math

import numpy as np

# Full-problem constants (hardcoded per the harness contract).
B_FULL = 16
N_CORES = 8
S_Q = 2048
S_KV = 2048
C_DIM = 1024  # input feature dim (contraction of the projections)
DK = 256  # q/k head dim
E_DIM = 1024  # v / output dim
P = 128
WSC = 8.0  # fp8 pre-scale on Wq/Wk/Wv (and the rowsum ones column)
CH = 512  # cast/transpose chunk rows
SBLK = 1024  # query columns per scores/AV wave


class CFG:
    def __init__(self, bl, sq, skv, c, dk, e, n_free=512):
        assert sq % P == 0 and skv % P == 0 and c % P == 0 and dk % P == 0
        self.bl = bl
        self.sq = sq
        self.skv = skv
        self.c = c
        self.dk = dk
        self.e = e
        self.n_free = n_free
        self.scale = 1.0 / math.sqrt(e)


def _chunks(total, size):
    out = []
    o = 0
    while o < total:
        out.append((o, min(size, total - o)))
        o += size
    return out


def emit_cross_attention(tc, outs, ins, cfg):
    """Emit the kernel into TileContext `tc`."""
    import concourse.mybir as mybir
    from concourse.mybir import ActivationFunctionType as AF
    from concourse.mybir import AluOpType as ALU
    from concourse.tile_rust import add_dep_helper

    DR = mybir.MatmulPerfMode.DoubleRow

    nc = tc.nc
    fp8 = mybir.dt.float8e4
    u16 = mybir.dt.uint16
    f32 = mybir.dt.float32

    x, y, Wq, Wk, Wv = ins["x"], ins["y"], ins["Wq"], ins["Wk"], ins["Wv"]
    out = outs["out"]

    n_ct = cfg.c // P  # 8 128-tiles of the projection contraction
    n_cg = n_ct // 2  # 4 DoubleRow groups
    n_tt = cfg.skv // P  # 16 key tiles
    n_tg = n_tt // 2  # 8 key DoubleRow groups
    n_dt = cfg.dk // P  # 2
    assert n_dt == 2
    NF = cfg.n_free
    n_waves = cfg.sq // SBLK  # 2
    mh_per_wave = SBLK // P  # 8
    tiles_per_chunk = CH // P  # 4

    # fp8 DRAM bounces of x and y.
    x8b = nc.dram_tensor("x8b", [cfg.bl, cfg.sq, cfg.c], fp8).ap()
    y8b = nc.dram_tensor("y8b", [cfg.bl, cfg.skv, cfg.c], fp8).ap()

    pool = tc.alloc_tile_pool(name="main", bufs=1)
    ps_mm = tc.alloc_tile_pool(name="ps_mm", bufs=3, space="PSUM")
    ps_av = tc.alloc_tile_pool(name="ps_av", bufs=2, space="PSUM")

    # ---------------- weights -------------------------------------------
    # k-pair weight tiles in the transposed-activation d-order:
    # w8[kc][p, j, :] = W[256*kc + 2p + j, :] * 8
    wq8 = [pool.tile([P, 2, cfg.dk], fp8, tag=f"wq{g}", name=f"wq{g}") for g in range(n_cg)]
    wk8 = [pool.tile([P, 2, cfg.dk], fp8, tag=f"wk{g}", name=f"wk{g}") for g in range(n_cg)]
    wv8 = [pool.tile([P, 2, cfg.e], fp8, tag=f"wv{g}", name=f"wv{g}") for g in range(n_cg)]

    def load_weight(which, w_dram, w_tiles, wdim, kc):
        w_f = pool.tile([P, 2, cfg.e], f32, tag="wstage", bufs=2, name=f"wf{which}{kc}")
        src = w_dram[kc * 2 * P : (kc + 1) * 2 * P, :].rearrange("(p j) w -> p j w", j=2)
        nc.scalar.dma_start(out=w_f[:, :, :wdim], in_=src)
        nc.scalar.activation(w_tiles[kc][:, :, :], w_f[:, :, :wdim], AF.Copy, scale=WSC)

    for kc in range(n_cg):
        load_weight("q", Wq, wq8, cfg.dk, kc)
    for kc in range(n_cg):
        load_weight("k", Wk, wk8, cfg.dk, kc)
    for kc in range(n_cg):
        load_weight("v", Wv, wv8, cfg.e, kc)

    ones_col = pool.tile([P, 2, 16], fp8, tag="ones", name="ones")
    nc.gpsimd.memset(ones_col[:], WSC)

    # ---------------- transposed fp8 activations ------------------------
    actT8 = {}
    for key in ("x0", "y0", "y1", "x1"):
        actT8[key] = [
            pool.tile([P, 2, cfg.skv], fp8, tag="actT8", bufs=3 * n_ct // 2,
                      name=f"T8_{key}_{g}")
            for g in range(n_cg)
        ]

    state = {"last_T": None}

    def pace(waiter, dependee):
        if waiter is not None and dependee is not None:
            add_dep_helper(waiter.ins, dependee.ins, sync=True, reason="pace dma windows")

    def cast_chunk(which, b, ro, rn):
        src = y if which == "y" else x
        dst = y8b if which == "y" else x8b
        c = nc.gpsimd.dma_start(out=dst[b][ro : ro + rn, :], in_=src[b][ro : ro + rn, :])
        pace(c, state["last_T"])
        return c

    def transpose_chunk(which, b, ro, rn):
        """u16-pair xbar transpose of bounce rows [ro:ro+rn] + DVE
        de-interleave into the fp8 k-pair tiles."""
        srcb = y8b if which == "y" else x8b
        stage = pool.tile([P, n_cg, CH], u16, tag="stage", bufs=3, name=f"st{which}{b}{ro}")
        t = nc.sync.dma_start(
            out=stage[:, :, :rn],
            in_=srcb[b][ro : ro + rn, :].bitcast(u16),
            transpose=True,
        )
        state["last_T"] = t
        st8 = stage.bitcast(fp8)  # [128, n_cg, 2*CH]
        dst = actT8[f"{which}{b}"]
        for kc in range(n_cg):
            nc.vector.tensor_copy(
                dst[kc][:, :, ro : ro + rn],
                st8[:, kc, : 2 * rn].rearrange("p (s j) -> p j s", j=2),
            )

    # ---------------- compute helpers -----------------------------------
    def proj_chunk(w8, src8, dst8, ro, rn):
        for md in range(n_dt):
            ps = ps_mm.tile([P, NF], f32, tag="mm", name="ps_p")
            for g in range(n_cg):
                nc.tensor.matmul(
                    ps[:, :rn],
                    w8[g][:, :, md * P : (md + 1) * P],
                    src8[g][:, :, ro : ro + rn],
                    start=(g == 0),
                    stop=(g == n_cg - 1),
                    perf_mode=DR,
                )
            nc.vector.tensor_copy(dst8[:, md, ro : ro + rn], ps[:, :rn])

    def v_mt(src8, v8b, mt):
        for no, nn_ in _chunks(cfg.e, NF):
            ps = ps_mm.tile([P, NF], f32, tag="mm", name="ps_v")
            for g in range(n_cg):
                nc.tensor.matmul(
                    ps[:, :nn_],
                    src8[g][:, :, mt * P : (mt + 1) * P],
                    wv8[g][:, :, no : no + nn_],
                    start=(g == 0),
                    stop=(g == n_cg - 1),
                    perf_mode=DR,
                )
            nc.vector.tensor_copy(v8b[mt // 2][:, mt % 2, no : no + nn_], ps[:, :nn_])

    s_scale = cfg.scale / (WSC * WSC)

    def scores_t(kT8b, qT8b, pT8, wo, t):
        for no, nn_ in _chunks(SBLK, NF):
            ps = ps_mm.tile([P, NF], f32, tag="mm", name="ps_s")
            nc.tensor.matmul(
                ps[:, :nn_],
                kT8b[:, :, t * P : (t + 1) * P],
                qT8b[:, :, wo + no : wo + no + nn_],
                start=True,
                stop=True,
                perf_mode=DR,
            )
            nc.scalar.activation(
                pT8[t // 2][:, t % 2, no : no + nn_], ps[:, :nn_], AF.Exp, scale=s_scale
            )

    def av_wave(b, pT8, v8b, wo, post_mh=None):
        ps_sum = ps_av.tile([P, mh_per_wave], f32, tag="av_s", bufs=1, name="ps_sum")
        recip = pool.tile([P, mh_per_wave], f32, tag="recip", bufs=2, name="recip")
        for mh in range(mh_per_wave):
            sm = wo + mh * P
            xres = pool.tile([P, cfg.e], f32, tag="xres", bufs=4, name="xres")
            nc.scalar.dma_start(out=xres[:], in_=x[b][sm : sm + P, :])
            ps_e = ps_av.tile([P, cfg.e], f32, tag="av_e", name="ps_e")
            e_chunks = _chunks(cfg.e, NF)
            for g in range(n_tg):
                lhsT = pT8[g][:, :, mh * P : (mh + 1) * P]
                for no, nn_ in e_chunks:
                    nc.tensor.matmul(
                        ps_e[:, no : no + nn_],
                        lhsT,
                        v8b[g][:, :, no : no + nn_],
                        start=(g == 0),
                        stop=(g == n_tg - 1),
                        perf_mode=DR,
                    )
                nc.tensor.matmul(
                    ps_sum[:, mh : mh + 1],
                    lhsT,
                    ones_col[:, :, 0:1],
                    start=(g == 0),
                    stop=(g == n_tg - 1),
                    perf_mode=DR,
                )
            nc.vector.reciprocal(recip[:, mh : mh + 1], ps_sum[:, mh : mh + 1])
            out_t = pool.tile([P, cfg.e], out.dtype, tag="out_t", bufs=6, name="out_t")
            nc.vector.scalar_tensor_tensor(
                out_t[:], ps_e[:], recip[:, mh : mh + 1], xres[:], ALU.mult, ALU.add
            )
            nc.scalar.dma_start(out=out[b][sm : sm + P, :], in_=out_t[:])
            if post_mh is not None:
                post_mh(mh)

    # ---------------- tiles ---------------------------------------------
    kT8 = {}
    qT8 = {}
    v8 = {}
    pT8 = {}
    for b in range(cfg.bl):
        kT8[b] = pool.tile([P, 2, cfg.skv], fp8, tag="kT", bufs=2, name=f"kT8_{b}")
        qT8[b] = pool.tile([P, 2, cfg.sq], fp8, tag="qT", bufs=2, name=f"qT8_{b}")
        v8[b] = [
            pool.tile([P, 2, cfg.e], fp8, tag="v", bufs=n_tg, name=f"v{b}_{g}")
            for g in range(n_tg)
        ]
        pT8[b] = {}
        for w in range(n_waves):
            pT8[b][w] = [
                pool.tile([P, 2, SBLK], fp8, tag="pT", bufs=2 * n_tg, name=f"pT{b}{w}_{g}")
                for g in range(n_tg)
            ]

    # ---------------- batch 0 prologue ----------------------------------
    # x chunks first (qT), then y chunks with kT + v + the scores panel.
    for ro, rn in _chunks(cfg.sq, CH):
        cast_chunk("x", 0, ro, rn)
        transpose_chunk("x", 0, ro, rn)
        proj_chunk(wq8, actT8["x0"], qT8[0], ro, rn)
    for ro, rn in _chunks(cfg.skv, CH):
        cast_chunk("y", 0, ro, rn)
        transpose_chunk("y", 0, ro, rn)
        proj_chunk(wk8, actT8["y0"], kT8[0], ro, rn)
        for mt in range(ro // P, (ro + rn) // P):
            v_mt(actT8["y0"], v8[0], mt)
        for t in range(ro // P, (ro + rn) // P):
            for w in range(n_waves):
                scores_t(kT8[0], qT8[0], pT8[0][w], w * SBLK, t)

    # b1 bounce casts bulk-stream now.
    cast_chunk("y", 1, 0, cfg.skv)
    cast_chunk("x", 1, 0, cfg.sq)

    # ---------------- b0 AV (+ b1 transpose sprinkles) ------------------
    def post_w0(mh):
        if mh in (1, 3, 5, 7):
            ci = (mh - 1) // 2
            transpose_chunk("y", 1, ci * CH, CH)

    def post_w1(mh):
        if mh in (1, 3, 5, 7):
            ci = (mh - 1) // 2
            transpose_chunk("x", 1, ci * CH, CH)

    av_wave(0, pT8[0][0], v8[0], 0, post_mh=post_w0)
    av_wave(0, pT8[0][1], v8[0], SBLK, post_mh=post_w1)

    # ---------------- batch 1 -------------------------------------------
    for ro, rn in _chunks(cfg.skv, CH):
        proj_chunk(wk8, actT8["y1"], kT8[1], ro, rn)
    for ro, rn in _chunks(cfg.sq, CH):
        proj_chunk(wq8, actT8["x1"], qT8[1], ro, rn)
    for w in range(n_waves):
        for t in range(n_tt):
            scores_t(kT8[1], qT8[1], pT8[1][w], w * SBLK, t)
            if t % 2 == 1:
                mt = w * (n_tt // n_waves) + t // 2
                v_mt(actT8["y1"], v8[1], mt)
    av_wave(1, pT8[1][0], v8[1], 0)
    av_wave(1, pT8[1][1], v8[1], SBLK)

    ps_av.release()
    ps_mm.release()
    pool.release()


def make_tile_kernel(cfg):
    def k(tc, outs, ins):
        emit_cross_attention(tc, outs, ins, cfg)

    return k


def _build(cfg):
    import concourse.bacc as bacc
    import concourse.mybir as mybir
    import concourse.tile as tile

    f32 = mybir.dt.float32
    bf16 = mybir.dt.bfloat16
    nc = bacc.Bacc(
        "TRN2",
        target_bir_lowering=False,
        debug=False,
        enable_asserts=False,
        num_devices=N_CORES,
    )
    ins = {
        "x": nc.dram_tensor("x", [cfg.bl, cfg.sq, cfg.c], f32, kind="ExternalInput").ap(),
        "y": nc.dram_tensor("y", [cfg.bl, cfg.skv, cfg.c], f32, kind="ExternalInput").ap(),
        "Wq": nc.dram_tensor("Wq", [cfg.c, cfg.dk], f32, kind="ExternalInput").ap(),
        "Wk": nc.dram_tensor("Wk", [cfg.c, cfg.dk], f32, kind="ExternalInput").ap(),
        "Wv": nc.dram_tensor("Wv", [cfg.c, cfg.e], f32, kind="ExternalInput").ap(),
    }
    outs = {
        "out": nc.dram_tensor("out", [cfg.bl, cfg.sq, cfg.e], bf16, kind="ExternalOutput").ap()
    }
    with tile.TileContext(nc) as tc:
        emit_cross_attention(tc, outs, ins, cfg)
    nc.compile()
    return nc


_CACHED = {}


def run_on_cores(x, y, Wq, Wk, Wv, trace=False):
    from concourse import bass_utils

    cfg = CFG(B_FULL // N_CORES, S_Q, S_KV, C_DIM, DK, E_DIM)
    key = "full"
    if key not in _CACHED:
        _CACHED[key] = _build(cfg)
    nc = _CACHED[key]

    bl = cfg.bl
    in_maps = [
        {
            "x": np.ascontiguousarray(x[i * bl : (i + 1) * bl]),
            "y": np.ascontiguousarray(y[i * bl : (i + 1) * bl]),
            "Wq": Wq,
            "Wk": Wk,
            "Wv": Wv,
        }
        for i in range(N_CORES)
    ]
    res = bass_utils.run_bass_kernel_spmd(
        nc, in_maps, core_ids=list(range(N_CORES)), trace=trace
    )
    out = np.concatenate(
        [np.asarray(r["out"], dtype=np.float32) for r in res.results], axis=0
    )
    return out, res


def kernel(x, y, Wq, Wk, Wv):
    x = np.asarray(x, dtype=np.float32)
    y = np.asarray(y, dtype=np.float32)
    Wq = np.asarray(Wq, dtype=np.float32)
    Wk = np.asarray(Wk, dtype=np.float32)
    Wv = np.asarray(Wv, dtype=np.float32)
    out, _ = run_on_cores(x, y, Wq, Wk, Wv, trace=False)
    return out
